# revision 1
# baseline (speedup 1.0000x reference)
"""Trainium2 Bass kernel for DPB (dynamic-position-bias) windowed attention.

Full inputs in, full outputs out. Shards data-parallel over the batch/window
axis across 8 NeuronCores (32 windows per core); all weights and the position
bias table are replicated per core.

Per-core pipeline for each window b (N=256 tokens, C=256 ch, 8 heads, d=32):
  1. qkT = (Wqk x_b)           PE fp32r, out (o, n); Act drains to bf16 SBUF
  2. v   = (x_b^T Wv)          PE fp32r; DVE-scattered into a column-packed
                               bf16 lhsT tile with den ones-columns
  3. st[m,n] = k_h^T q_h       PE bf16, K=32 row-packed, no bias preload
  4. P~ = exp(st)              ScalarE, PSUM->SBUF bf16 (big per-window tile)
  5. P  = P~ * exp(rpb)        DVE/Pool scalar_tensor_tensor (4x mode);
                               position bias folded in as a host-precomputed
                               multiplicative table
  6. av/den: one M=128 matmul per (head, mc) with column-packed lhsT
     [v_e|0|1|0] / [0|v_o|0|1] -> av rows 0..63, den rows 64..127
     (denominator rides the AV matmul for free; cost is FD-bound)
  7. outT = av * recip(den)    DVE (or direct divide with KERNEL_DIV=1)
  8. final = Wproj outT        PE fp32r, K=2x128; DVE drain, DMA out
"""

import os
import sys

sys.path.insert(0, "/opt/trn_rl_repo")

import ml_dtypes
import numpy as np
from contextlib import ExitStack

import concourse.bacc as bacc
import concourse.tile as tile
from concourse import mybir
from concourse.bass_utils import run_bass_kernel_spmd

F32 = mybir.dt.float32
F32R = mybir.dt.float32r
BF16 = mybir.dt.bfloat16
EXP = mybir.ActivationFunctionType.Exp
MULT = mybir.AluOpType.mult
BYPASS = mybir.AluOpType.bypass
DIVIDE = mybir.AluOpType.divide

NUM_HEADS = 8
B, C, H, W = 256, 256, 16, 16
N = H * W            # 256 tokens per window
D = C // NUM_HEADS   # 32
PD = 16              # pos-MLP hidden dim
LN_EPS = 1e-5
NCORES = 8
BPC = int(os.environ.get("KERNEL_BPC", B // NCORES))  # windows per core (32)
USE_DIV = os.environ.get("KERNEL_DIV", "0") == "1"


# ---------------------------------------------------------------- host math
def _rel_index():
    coords = np.stack(np.meshgrid(np.arange(H), np.arange(W), indexing="ij"))
    cf = coords.reshape(2, -1)
    rel = (cf[:, :, None] - cf[:, None, :]).transpose(1, 2, 0)
    rel = rel + np.array([H - 1, W - 1])
    rel[..., 0] *= 2 * W - 1
    return rel.sum(-1)  # (N, N) int


def _biases():
    bh = np.arange(1 - H, H)
    bw = np.arange(1 - W, W)
    b = np.stack(np.meshgrid(bh, bw, indexing="ij")).reshape(2, -1).T
    return b.astype(np.float32)  # ((2H-1)(2W-1), 2)


def _ln(x, g, b):
    mu = x.mean(-1, keepdims=True)
    var = ((x - mu) ** 2).mean(-1, keepdims=True)
    return (x - mu) / np.sqrt(var + LN_EPS) * g + b


def _pos_mlp(pos_proj_w, pos_proj_b, ln1_g, ln1_b, fc1_w, fc1_b,
             ln2_g, ln2_b, fc2_w, fc2_b, ln3_g, ln3_b, fc3_w, fc3_b):
    p = _biases() @ pos_proj_w.T + pos_proj_b
    p = np.maximum(_ln(p, ln1_g, ln1_b), 0.0) @ fc1_w.T + fc1_b
    p = np.maximum(_ln(p, ln2_g, ln2_b), 0.0) @ fc2_w.T + fc2_b
    p = np.maximum(_ln(p, ln3_g, ln3_b), 0.0) @ fc3_w.T + fc3_b
    return p.astype(np.float32)  # (L, heads)


# ------------------------------------------------------------- bass program
_PROGRAM_CACHE = {}
_LAST_RESULTS = None


def _build_program(has_qkvb, has_projb):
    key = (has_qkvb, has_projb, USE_DIV,
           os.environ.get("KERNEL_RECMODE", "0"),
           os.environ.get("KERNEL_DEBUG", "0"),
           os.environ.get("KERNEL_REPS", "1"),
           os.environ.get("KERNEL_UNROLL", "1"),
           os.environ.get("KERNEL_ERPB", "0"))
    if key in _PROGRAM_CACHE:
        return _PROGRAM_CACHE[key]

    nc = bacc.Bacc("TRN2", target_bir_lowering=False, debug=False)
    N_ERPB = int(os.environ.get("KERNEL_ERPB", "0"))

    def din(name, shp, dt=F32R):
        return nc.dram_tensor(name, shp, dt, kind="ExternalInput").ap()

    x_d = din("x", [BPC, 128, 2, N])            # per-core x, (b, c_p, cc, n)
    wqk_d = din("wqk", [2, 128, 512])           # (c-chunk, c_p, o) q|k, q scaled
    wv_d = din("wv", [2, 128, 256])             # (c-chunk, c_p, o_v)
    wp_d = din("wp", [128, 2, 256])             # proj weights (c_p, kc, o)
    rpb_d = din("rpb", [128, NUM_HEADS, 2, 256])     # bias table (p, h, mc, n)
    erpb_d = din("erpb", [128, 4, 2, 2, 256], BF16)  # exp(rpb) (p, pg, par, mc, n)
    ident_d = din("ident", [128, 128])
    vpre_d = din("vpre", [128, 2, 4, 2, 128], BF16)  # v' zeros/ones prefill
    zout_d = din("zout", [128, 2, 2, 256])
    qkvb_d = din("qkvb", [1, 768])
    projb_d = din("projb", [1, 256])
    onesr_d = din("onesr", [1, 512])
    out_d = nc.dram_tensor("out", [BPC, 128, 2, N], F32, kind="ExternalOutput").ap()
    DEBUG = bool(int(os.environ.get("KERNEL_DEBUG", "0")))
    if DEBUG:
        dbg_qk = nc.dram_tensor("dbg_qk", [128, 4, 256], F32, kind="ExternalOutput").ap()
        dbg_v = nc.dram_tensor("dbg_v", [128, 2, 4, 2, 128], F32, kind="ExternalOutput").ap()
        dbg_p = nc.dram_tensor("dbg_p", [128, 2, 2, 256], F32, kind="ExternalOutput").ap()
        dbg_av = nc.dram_tensor("dbg_av", [128, 2, 256], F32, kind="ExternalOutput").ap()
        dbg_outT = nc.dram_tensor("dbg_outT", [128, 2, 256], F32, kind="ExternalOutput").ap()

    with tile.TileContext(nc) as tc, ExitStack() as ctx:
        consts = ctx.enter_context(tc.tile_pool(name="consts", bufs=1))
        px = ctx.enter_context(tc.tile_pool(name="px", bufs=4))
        pqk = ctx.enter_context(tc.tile_pool(name="pqk", bufs=4))
        pp = ctx.enter_context(tc.tile_pool(name="pp", bufs=4))
        pp2 = ctx.enter_context(tc.tile_pool(name="pp2", bufs=3))
        prec = ctx.enter_context(tc.tile_pool(name="prec", bufs=4))
        pout = ctx.enter_context(tc.tile_pool(name="pout", bufs=4))
        pdbg = (ctx.enter_context(tc.tile_pool(name="pdbg", bufs=1))
                if bool(int(os.environ.get("KERNEL_DEBUG", "0"))) else None)
        # PSUM: st pair-tiles 2 banks x bufs=2 (4 banks) + shared 1-bank pool
        # bufs=4 (4 banks) = all 8 banks.
        pst = ctx.enter_context(tc.tile_pool(name="pst", bufs=2, space="PSUM"))
        psh = ctx.enter_context(tc.tile_pool(name="psh", bufs=4, space="PSUM"))

        wqk_s = consts.tile([128, 2, 512], F32R)
        nc.sync.dma_start(wqk_s[:], wqk_d.transpose([1, 0, 2]))
        wv_s = consts.tile([128, 2, 256], F32R)
        nc.sync.dma_start(wv_s[:], wv_d.transpose([1, 0, 2]))
        wp_s = consts.tile([128, 2, 256], F32R)
        nc.sync.dma_start(wp_s[:], wp_d[:])
        rpb_s = consts.tile([128, NUM_HEADS, 2, 256], F32R)
        nc.sync.dma_start(rpb_s[:], rpb_d[:])
        erpb_s = consts.tile([128, 4, 2, 2, 256], BF16)
        nc.sync.dma_start(erpb_s[:], erpb_d[:])
        ident_s = consts.tile([128, 128], F32R)
        nc.sync.dma_start(ident_s[:], ident_d[:])
        qkvb_s = consts.tile([1, 768], F32R)
        nc.sync.dma_start(qkvb_s[:], qkvb_d[:])
        projb_s = consts.tile([1, 256], F32R)
        nc.sync.dma_start(projb_s[:], projb_d[:])
        onesr_s = consts.tile([1, 512], F32R)
        nc.sync.dma_start(onesr_s[:], onesr_d[:])

        # persistent v' tiles (alternating per window parity): zeros/ones
        # column packing prefilled once; only the v columns are rewritten.
        # layout [128(m), 2(mc), 4(pair), 2(eo), 128]; pair block cols:
        #   eo=0: [v_e(0:32) | 0 | ones(64:96) | 0]
        #   eo=1: [0 | v_o(32:64) | 0 | ones(96:128)]
        vt = [consts.tile([128, 2, 4, 2, 128], BF16, name=f"vt{i}", tag=f"vt{i}")
              for i in range(2)]
        nc.sync.dma_start(vt[0][:], vpre_d[:])
        nc.sync.dma_start(vt[1][:], vpre_d[:])

        # persistent outT buffers (alternating per batch-pair parity),
        # packed: kc-chunk kc holds heads 4kc..4kc+3 (32 rows each)
        outT = [consts.tile([128, 2, 2, 256], F32R, name=f"outT{i}", tag=f"outT{i}")
                for i in range(2)]
        nc.sync.dma_start(outT[0][:], zout_d[:])
        nc.sync.dma_start(outT[1][:], zout_d[:])

        def emit_proj(pb):
            f_psl = [psh.tile([128, 2, 256], F32, tag="bank", name=f"fp{oc}")
                     for oc in range(2)]
            for oc in range(2):
                osl = slice(oc * 128, oc * 128 + 128)
                for kc in range(2):
                    nc.tensor.matmul(f_psl[oc][:], wp_s[:, kc, osl],
                                     outT[pb % 2][:, kc, :, :],
                                     start=(kc == 0),
                                     stop=(kc == 1 and not has_projb))
                if has_projb:
                    nc.tensor.matmul(f_psl[oc][:], projb_s[:, osl],
                                     onesr_s[:, :], start=False, stop=True)
            out_sb = pout.tile([128, 2, 2, 256], F32, tag="out")
            for oc in range(2):
                nc.vector.tensor_copy(out_sb[:, :, oc, :], f_psl[oc][:])
            for b01 in range(2):
                nc.gpsimd.dma_start(out_d[2 * pb + b01], out_sb[:, b01, :, :])

        reps = int(os.environ.get("KERNEL_REPS", "1"))
        unroll = int(os.environ.get("KERNEL_UNROLL", "1"))
        rep_ctx = tc.For_i(0, reps, 1) if reps > 1 else None
        if rep_ctx is not None:
            rep_ctx.__enter__()
        for bp in [b for _ in range(unroll) for b in range(BPC // 2)]:
            x_sb = px.tile([128, 2, 2, 256], F32R, tag="x")
            for b01 in range(2):
                nc.sync.dma_start(x_sb[:, b01, :, :], x_d[2 * bp + b01])

            # ---- qk projection, both windows at FD=512: out (o, [b0|b1] n)
            qk_psl = [psh.tile([128, 512], F32, tag="bank", name=f"qkp{oc}")
                      for oc in range(4)]
            for oc in range(4):
                osl = slice(oc * 128, oc * 128 + 128)
                nc.tensor.matmul(qk_psl[oc][:], wqk_s[:, 0, osl],
                                 x_sb[:, :, 0, :], start=True, stop=False)
                nc.tensor.matmul(qk_psl[oc][:], wqk_s[:, 1, osl],
                                 x_sb[:, :, 1, :], start=False, stop=not has_qkvb)
                if has_qkvb:
                    nc.tensor.matmul(qk_psl[oc][:], qkvb_s[:, osl], onesr_s[:, :],
                                     start=False, stop=True)
            # PSUM -> SBUF bf16 drains on Act
            qk_sb = pqk.tile([128, 4, 2, 256], BF16, tag="qk")
            for oc in range(4):
                nc.scalar.copy(qk_sb[:, oc, :, :],
                               qk_psl[oc][:].rearrange("p (b n) -> p b n", b=2))
            if DEBUG and bp == 0:
                dqk = pdbg.tile([128, 4, 256], F32, tag="dbgqk")
                nc.vector.tensor_copy(dqk[:], qk_sb[:, :, 0, :])
                nc.sync.dma_start(dbg_qk[:], dqk[:])

            # ---- output projection for the PREVIOUS bp (software
            # pipelined; issued early so its PSUM tiles drain early)
            emit_proj((bp - 1) % (BPC // 2))

            # ---- v projections, both windows (stage-major order)
            for b01 in range(2):
                b = 2 * bp + b01
                vts = vt[b01]
                v_ps = psh.tile([128, 2, 4, 64], F32, tag="bank")
                for mc in range(2):
                    nc.tensor.matmul(v_ps[:, mc, :, :],
                                     x_sb[:, b01, 0, mc * 128:mc * 128 + 128],
                                     wv_s[:, 0, :], start=True, stop=False)
                    nc.tensor.matmul(v_ps[:, mc, :, :],
                                     x_sb[:, b01, 1, mc * 128:mc * 128 + 128],
                                     wv_s[:, 1, :], start=False, stop=not has_qkvb)
                    if has_qkvb:
                        nc.tensor.matmul(v_ps[:, mc, :, :], onesr_s[:, 0:128],
                                         qkvb_s[:, 512:768], start=False, stop=True)
                # scatter v columns into the prefilled v' tile (DVE)
                nc.vector.tensor_copy(vts[:, :, :, 0, 0:32], v_ps[:, :, :, 0:32])
                nc.vector.tensor_copy(vts[:, :, :, 1, 32:64], v_ps[:, :, :, 32:64])
                if DEBUG and b == 0:
                    dv = pdbg.tile([128, 2, 4, 2, 128], F32, tag="dbgv")
                    nc.vector.tensor_copy(dv[:], vts[:])
                    nc.sync.dma_start(dbg_v[:], dv[:])

            # ---- attention scores + exp + bias, both windows
            p_sbs = []
            for b01 in range(2):
                b = 2 * bp + b01
                p_sb = pp2.tile([128, 4, 2, 2, 256], BF16, tag="p")
                p_sbs.append(p_sb)
                for pg in range(4):  # heads (2pg, 2pg+1)
                    use_erpb = pg < N_ERPB
                    st = pst.tile([128, 2, 2, 256], F32, tag="st")
                    if not use_erpb:
                        for par in range(2):
                            h = 2 * pg + par
                            nc.tensor.matmul(st[:, par, :, :], ident_s[:, :],
                                             rpb_s[:, h, :, :], start=True,
                                             stop=False)
                    for par in range(2):
                        h = 2 * pg + par
                        hl = h % 4
                        hsl = slice(hl * 32, hl * 32 + 32)
                        qoc = h // 4
                        koc = 2 + h // 4
                        for mc in range(2):
                            nc.tensor.matmul(
                                st[:, par, mc, :],
                                qk_sb[hsl, koc, b01, mc * 128:mc * 128 + 128],
                                qk_sb[hsl, qoc, b01, :],
                                start=use_erpb, stop=(mc == 1),
                                tile_position=(hl * 32, 0))
                    if use_erpb:
                        pt = pp.tile([128, 2, 2, 256], BF16, tag="pt")
                        nc.scalar.activation(pt[:], st[:], EXP)
                        nc.vector.tensor_mul(p_sb[:, pg, :, :, :], pt[:],
                                             erpb_s[:, pg, :, :, :])
                    else:
                        nc.scalar.activation(p_sb[:, pg, :, :, :], st[:], EXP)
                if DEBUG and b == 0:
                    dp = pdbg.tile([128, 2, 2, 256], F32, tag="dbgp")
                    nc.vector.tensor_copy(dp[:], p_sb[:, 0])
                    nc.sync.dma_start(dbg_p[:], dp[:])

            # ---- av + den + normalize, both windows
            for b01 in range(2):
                b = 2 * bp + b01
                vts = vt[b01]
                p_sb = p_sbs[b01]
                av_banks = []
                for g in range(2):
                    av_ps = psh.tile([128, 2, 256], F32, tag="bank")
                    av_banks.append(av_ps)
                    for t in range(2):
                        pg = 2 * g + t
                        for par in range(2):
                            h = 2 * pg + par
                            for mc in range(2):
                                nc.tensor.matmul(
                                    av_ps[:, t, :],
                                    vts[:, mc, h // 2, h % 2, :],
                                    p_sb[:, pg, par, mc, :],
                                    start=(par == 0 and mc == 0),
                                    stop=(par == 1 and mc == 1))

                for g in range(2):  # av bank g -> outT kc chunk g
                    av_ps = av_banks[g]
                    if USE_DIV:
                        for t in range(2):
                            nc.vector.tensor_tensor(
                                outT[bp % 2][64 * t:64 * t + 64, g, b01, :],
                                av_ps[0:64, t, :], av_ps[64:128, t, :], DIVIDE)
                    else:
                        rec = prec.tile([128, 2, 256], F32, tag="rec")
                        nc.vector.reciprocal(rec[64:128, :, :],
                                             av_ps[64:128, :, :])
                        for t in range(2):
                            nc.vector.tensor_mul(
                                outT[bp % 2][64 * t:64 * t + 64, g, b01, :],
                                av_ps[0:64, t, :], rec[64:128, t, :])
                    if DEBUG and b == 0 and g == 0:
                        dav = pdbg.tile([128, 2, 256], F32, tag="dbgav")
                        nc.vector.tensor_copy(dav[:], av_ps[:])
                        nc.sync.dma_start(dbg_av[:], dav[:])
                if DEBUG and b == 0:
                    dot = pdbg.tile([128, 2, 256], F32, tag="dbgot")
                    nc.vector.tensor_copy(dot[:], outT[0].bitcast(F32)[:, :, 0, :])
                    nc.sync.dma_start(dbg_outT[:], dot[:])
        emit_proj(BPC // 2 - 1)  # tail flush (overwrites the bogus bp=-1 run)
        if rep_ctx is not None:
            rep_ctx.__exit__(None, None, None)

    nc.compile()
    _PROGRAM_CACHE[key] = nc
    return nc


# ------------------------------------------------------------------ kernel
def host_prep(inp):
    """Returns (in_maps, has_qkvb, has_projb) for the 8 cores."""
    x = inp["x"].reshape(B, C, N)
    scale = float(D) ** -0.5

    # fold the attention scale into the q columns of qkv_w
    qkv_wT = inp["qkv_w"].T.copy()            # (c, o)
    qkv_wT[:, 0:C] *= scale
    wqk = np.ascontiguousarray(
        qkv_wT[:, 0:512].reshape(2, 128, 512))
    wv = np.ascontiguousarray(
        qkv_wT[:, 512:768].reshape(2, 128, 256))

    # proj weights: (c_p, kc, o); outT kc-chunk kc = heads 4kc..4kc+3 packed
    # in natural c order, so this is just a reshape of proj_w^T.
    proj_wT = inp["proj_w"].T.copy()          # (c, o)
    wp = np.ascontiguousarray(
        proj_wT.reshape(2, 128, 256).transpose(1, 0, 2))

    # dynamic position bias table via the tiny MLP (host, fp32), exp'd for
    # multiplicative application after exp(st)
    p_mlp = _pos_mlp(
        inp["pos_proj_w"], inp["pos_proj_b"], inp["ln1_g"], inp["ln1_b"],
        inp["fc1_w"], inp["fc1_b"], inp["ln2_g"], inp["ln2_b"],
        inp["fc2_w"], inp["fc2_b"], inp["ln3_g"], inp["ln3_b"],
        inp["fc3_w"], inp["fc3_b"])          # (L, heads)
    idx = _rel_index()                        # (N, N): bias[h][n, m]
    rpb_full = p_mlp[idx]                     # (n, m, heads)
    # rpb[p, h, mc, n] = rpb_full[n, mc*128+p, h]  (transposed layout)
    rpb = np.ascontiguousarray(
        rpb_full.transpose(1, 0, 2).reshape(2, 128, N, NUM_HEADS)
        .transpose(1, 0, 2, 3)               # (p, mc, n, h)
        .transpose(0, 3, 1, 2))              # (p, h, mc, n)
    erpb = np.ascontiguousarray(
        np.exp(rpb).reshape(128, 4, 2, 2, 256)).astype(ml_dtypes.bfloat16)

    # v' prefill [128, 2(mc), 4(pair), 2(eo), 128]: zeros + den ones-columns
    vpre = np.zeros((128, 2, 4, 2, 128), np.float32)
    vpre[:, :, :, 0, 64:96] = 1.0
    vpre[:, :, :, 1, 96:128] = 1.0
    vpre = vpre.astype(ml_dtypes.bfloat16)

    qkvb = inp["qkv_b"].reshape(1, 768).copy()
    qkvb[0, 0:C] *= scale
    projb = inp["proj_b"].reshape(1, 256)
    has_qkvb = bool(np.any(qkvb))
    has_projb = bool(np.any(projb))

    shared = {
        "wqk": wqk, "wv": wv, "wp": wp, "rpb": rpb, "erpb": erpb,
        "vpre": vpre, "zout": np.zeros((128, 2, 2, 256), np.float32),
        "ident": np.eye(128, dtype=np.float32),
        "qkvb": qkvb, "projb": projb,
        "onesr": np.ones((1, 512), np.float32),
    }
    # x partition-major per core: (b, c_p, cc, n)
    x_pm = np.ascontiguousarray(
        x.reshape(B, 2, 128, N).transpose(0, 2, 1, 3))
    in_maps = [
        {"x": x_pm[c * BPC:(c + 1) * BPC], **shared}
        for c in range(NCORES)
    ]
    return in_maps, has_qkvb, has_projb


def kernel(**inputs):
    inp = {k: np.asarray(v, dtype=np.float32) for k, v in inputs.items()}
    in_maps, has_qkvb, has_projb = host_prep(inp)
    nc = _build_program(has_qkvb, has_projb)
    res = run_bass_kernel_spmd(nc, in_maps, list(range(NCORES)))
    global _LAST_RESULTS
    _LAST_RESULTS = res.results
    out = np.concatenate([res.results[c]["out"] for c in range(NCORES)], axis=0)
    out = out.transpose(0, 2, 1, 3).reshape(-1, C, H, W)
    return np.ascontiguousarray(out).astype(np.float32)


if __name__ == "__main__":
    rng = np.random.default_rng(0)
    demo = {"x": rng.standard_normal((B, C, H, W), dtype=np.float32)}
    print("kernel module loaded")

